# revision 18
# baseline (speedup 1.0000x reference)
"""BSI quantized linear kernel for Trainium2 (8 NeuronCores, SPMD).

Computes out = round(x*100)/100 @ (round(W*100)/100).T + b for
x [4096, 4096] f32, W [4096, 4096] f32, b [4096] f32.

Sharding: 2x4 grid. x rows split 2 ways (2048 rows/core, 32MB), W
out_features split 4 ways (1024 rows/core, 16MB). Each core emits a
[2048, 1024] f32 block; the host assembles the grid. 56MB HBM/core.

Math: quantized values round(100*v) are small integers (|x| <= ~550,
|W| <= ~11), exact in fp16. GEMM runs fp16 on the PE, f32 PSUM
accumulation, then 1e-4 scale + bias. Rounding via the fp32
magic-number trick (+1.5*2^23, round-half-to-even) = jnp.round here.

Resource plan (k must sit on partitions for the PE):
  - x: half-stripe loads on the SP DMA queue -> DVE magic-round ->
    ACT fp16 convert -> one full-stripe XBAR DMA transpose per stripe
    on the ACT queue (XBARs must not run concurrently on two queues;
    they corrupt).
  - W: loads ride the GpSimd queue (emitted first so the preamble
    overlaps the x ramp), quantize entirely on DVE (tensor_scalar +
    f32->f16 copy) in dedicated pools so the x stream never blocks,
    PE-transposed (256 blocks), staged and published per o-chunk with
    a single copy.
  - out: ACT 1e-4 psum drain, DVE bias add, stores on the GpSimd queue.
"""

import numpy as np

_B, _D, _DOUT = 4096, 4096, 4096
_NCORES = 8
_R = 2                    # x row shards
_C = 4                    # W out_feature shards
_BPER = _B // _R          # 2048 x rows per core
_OPER = _DOUT // _C       # 1024 out features per core
_MAGIC = 12582912.0       # 1.5 * 2**23
_P = 128

_nc_cache = {}


def _build(BPER, D, OPER):
    import concourse.mybir as mybir
    import concourse.tile as tile
    from concourse import bacc
    from concourse.masks import make_identity

    f32 = mybir.dt.float32
    f16 = mybir.dt.float16
    Copy = mybir.ActivationFunctionType.Copy
    P = _P
    KT = D // P           # 32 k tiles
    BT = BPER // P        # 16 x stripes
    OT = OPER // P        # 8 W row-tiles
    NOC = 512             # matmul free size / PSUM bank
    OC = OPER // NOC      # 2 o-chunks
    HW_ = D // 2          # half-stripe width (2048)

    nc = bacc.Bacc("TRN2", target_bir_lowering=False, debug=False,
                   num_devices=_NCORES)
    x_d = nc.dram_tensor("x", [BPER, D], f32, kind="ExternalInput").ap()
    w_d = nc.dram_tensor("w", [OPER, D], f32, kind="ExternalInput").ap()
    b_d = nc.dram_tensor("b", [OPER], f32, kind="ExternalInput").ap()
    o_d = nc.dram_tensor("out", [BPER, OPER], f32, kind="ExternalOutput").ap()

    with tile.TileContext(nc) as tc:
        with (
            tc.tile_pool(name="const", bufs=1) as cpool,
            tc.tile_pool(name="wq", bufs=1) as wpool,
            tc.tile_pool(name="wstg", bufs=1) as wstgpool,
            tc.tile_pool(name="wload", bufs=2) as wlpool,
            tc.tile_pool(name="wq16", bufs=2) as wqpool,
            tc.tile_pool(name="stage", bufs=3) as spool,
            tc.tile_pool(name="q16", bufs=2) as qpool,
            tc.tile_pool(name="xT", bufs=3) as xtpool,
            tc.tile_pool(name="tps", bufs=2, space="PSUM") as tppool,
            tc.tile_pool(name="mmps", bufs=4, space="PSUM") as mmpool,
            tc.tile_pool(name="osb", bufs=2) as opool,
        ):
            ident = cpool.tile([P, P], f16)
            make_identity(nc, ident)
            bias_bc = cpool.tile([P, OPER], f32)
            nc.sync.dma_start(bias_bc, b_d[None, :].to_broadcast((P, OPER)))

            # Quantized transposed W, SBUF resident: one tile per o-chunk,
            # each [128, KT, NOC] fp16, published by a single copy.
            wT = [wpool.tile([P, KT, NOC], f16, name=f"wT{oc}")
                  for oc in range(OC)]
            wstage = wstgpool.tile([P, KT, NOC], f16)

            # ---- W preamble (dedicated pools, GpSimd queue, DVE quantize)
            for oc in range(OC):
                for oi in range(OT // OC):
                    ot = oc * (OT // OC) + oi
                    wq = wqpool.tile([P, D], f16, tag="wq16")
                    weng = nc.sync if oc == 0 else nc.scalar
                    for h in range(2):
                        wl = wlpool.tile([P, HW_], f32, tag="wload")
                        weng.dma_start(
                            wl, w_d[ot * P:(ot + 1) * P,
                                    h * HW_:(h + 1) * HW_])
                        nc.vector.tensor_scalar(wl, wl, 100.0, _MAGIC,
                                                mybir.AluOpType.mult,
                                                mybir.AluOpType.add)
                        nc.vector.tensor_scalar(
                            wq[:, h * HW_:(h + 1) * HW_], wl,
                            -_MAGIC, 0.0,
                            mybir.AluOpType.add, mybir.AluOpType.add)
                    for g in range(KT // 8):
                        tp = tppool.tile([P, 8, P], f16, tag="tps")
                        for j in range(8):
                            kt = g * 8 + j
                            nc.tensor.transpose(tp[:, j, :],
                                                wq[:, kt * P:(kt + 1) * P],
                                                ident)
                        nc.vector.tensor_copy(
                            wstage[:, g * 8:(g + 1) * 8, oi * P:(oi + 1) * P],
                            tp)
                nc.vector.tensor_copy(wT[oc], wstage)

            # ---- Main loop over x stripes
            for bt in range(BT):
                q = qpool.tile([P, D], f16, tag="q16")
                for h in range(2):
                    st = spool.tile([P, HW_], f32, tag="stage")
                    nc.sync.dma_start(st, x_d[bt * P:(bt + 1) * P,
                                              h * HW_:(h + 1) * HW_])
                    nc.vector.tensor_scalar(st, st, 100.0, _MAGIC,
                                            mybir.AluOpType.mult,
                                            mybir.AluOpType.add)
                    nc.scalar.activation(q[:, h * HW_:(h + 1) * HW_], st,
                                         Copy, bias=-_MAGIC, scale=1.0)
                xT = xtpool.tile([P, KT, P], f16, tag="xT")
                # Full-stripe XBAR transpose [128, 4096] -> [4096, 128],
                # landing as xT[:, kt, :] (k = kt*128 + partition).
                nc.scalar.dma_start(xT, q, transpose=True)

                ob = opool.tile([P, OPER], f32, tag="osb")
                for oc in range(OC):
                    ps = mmpool.tile([P, NOC], f32, name="ps", tag="mmps")
                    for kt in range(KT):
                        nc.tensor.matmul(ps, xT[:, kt, :], wT[oc][:, kt, :],
                                         start=(kt == 0), stop=(kt == KT - 1))
                    nc.scalar.activation(ob[:, oc * NOC:(oc + 1) * NOC],
                                         ps, Copy, bias=0.0, scale=1e-4)
                nc.vector.tensor_add(ob, ob, bias_bc)
                nc.gpsimd.dma_start(o_d[bt * P:(bt + 1) * P, :], ob)

    nc.compile()
    return nc


def _get_nc(BPER=_BPER, D=_D, OPER=_OPER):
    key = (BPER, D, OPER)
    if key not in _nc_cache:
        _nc_cache[key] = _build(BPER, D, OPER)
    return _nc_cache[key]


def _make_in_maps(x, W, b):
    BPER = x.shape[0] // _R
    OPER = W.shape[0] // _C
    in_maps = []
    for c in range(_NCORES):
        r, q = divmod(c, _C)
        in_maps.append({
            "x": np.ascontiguousarray(x[r * BPER:(r + 1) * BPER]),
            "w": np.ascontiguousarray(W[q * OPER:(q + 1) * OPER]),
            "b": np.ascontiguousarray(b[q * OPER:(q + 1) * OPER]),
        })
    return in_maps


def _assemble(outs):
    rows = []
    for r in range(_R):
        rows.append(np.concatenate(outs[r * _C:(r + 1) * _C], axis=1))
    return np.concatenate(rows, axis=0)


def _run(x, W, b, trace=False):
    from concourse.bass_utils import run_bass_kernel_spmd

    BPER = x.shape[0] // _R
    D = x.shape[1]
    OPER = W.shape[0] // _C
    nc = _get_nc(BPER, D, OPER)
    in_maps = _make_in_maps(x, W, b)
    res = run_bass_kernel_spmd(nc, in_maps, core_ids=list(range(_NCORES)),
                               trace=trace)
    out = _assemble([res.results[c]["out"] for c in range(_NCORES)])
    return out, res


def kernel(x=None, W=None, b=None):
    x = np.ascontiguousarray(np.asarray(x, dtype=np.float32))
    W = np.ascontiguousarray(np.asarray(W, dtype=np.float32))
    b = np.ascontiguousarray(np.asarray(b, dtype=np.float32))
    out, _ = _run(x, W, b, trace=False)
    return out


# revision 20
# speedup vs baseline: 1.0174x; 1.0174x over previous
"""BSI quantized linear kernel for Trainium2 (8 NeuronCores, SPMD).

Computes out = round(x*100)/100 @ (round(W*100)/100).T + b for
x [4096, 4096] f32, W [4096, 4096] f32, b [4096] f32.

Sharding: 2x4 grid. x rows split 2 ways (2048 rows/core, 32MB), W
out_features split 4 ways (1024 rows/core, 16MB). Each core emits a
[2048, 1024] f32 block; the host assembles the grid. 56MB HBM/core
(vs 80MB for 1x8 column-parallel with x replicated).

Math: quantized values round(100*v) are small integers (|x| <= ~550,
|W| <= ~11), exact in fp16. The GEMM runs fp16 on the PE at full rate,
accumulating exact integer dots in f32 PSUM, then 1e-4 scale + bias.
Rounding uses the fp32 magic-number trick (+1.5*2^23,
round-half-to-even), matching jnp.round bit-for-bit on this grid.

Per-core pipeline (B-row stripes of 128, PE-transposed x):
  DMA(SP)   x stripe f32
  DVE       t = 100*x + MAGIC       (f32, in place)
  ACT       q = t - MAGIC -> fp16   (integer-valued fp16)
  PE        transpose 128x128 q blocks -> PSUM; DVE copies -> xT
  PE        2 o-chunks x 32-step K accumulation vs resident wT halves
  ACT       osb = 1e-4 * psum;  DVE osb += bias
  DMA(GpSimd) out stripe
W rides the ACT DMA queue (so it loads in parallel with the x stream
during ramp), is quantized DVE-only, PE-transposed, and published per
512-wide o-chunk with a single copy each.
"""

import numpy as np

_B, _D, _DOUT = 4096, 4096, 4096
_NCORES = 8
_R = 2                    # x row shards
_C = 4                    # W out_feature shards
_BPER = _B // _R          # 2048 x rows per core
_OPER = _DOUT // _C       # 1024 out features per core
_MAGIC = 12582912.0       # 1.5 * 2**23
_P = 128

_nc_cache = {}


def _build(BPER, D, OPER):
    import concourse.mybir as mybir
    import concourse.tile as tile
    from concourse import bacc
    from concourse.masks import make_identity

    f32 = mybir.dt.float32
    f16 = mybir.dt.float16
    Copy = mybir.ActivationFunctionType.Copy
    P = _P
    KT = D // P           # 32 k tiles
    BT = BPER // P        # 16 x stripes
    OT = OPER // P        # 8 W row-tiles
    NOC = 512             # matmul free size / PSUM bank
    OC = OPER // NOC      # 2 o-chunks
    HW_ = D // 2          # half width for W loads

    nc = bacc.Bacc("TRN2", target_bir_lowering=False, debug=False,
                   num_devices=_NCORES)
    x_d = nc.dram_tensor("x", [BPER, D], f32, kind="ExternalInput").ap()
    w_d = nc.dram_tensor("w", [OPER, D], f32, kind="ExternalInput").ap()
    b_d = nc.dram_tensor("b", [OPER], f32, kind="ExternalInput").ap()
    o_d = nc.dram_tensor("out", [BPER, OPER], f32, kind="ExternalOutput").ap()

    with tile.TileContext(nc) as tc:
        with (
            tc.tile_pool(name="const", bufs=1) as cpool,
            tc.tile_pool(name="wq", bufs=1) as wpool,
            tc.tile_pool(name="wstg", bufs=1) as wstgpool,
            tc.tile_pool(name="wload", bufs=2) as wlpool,
            tc.tile_pool(name="wq16", bufs=1) as wqpool,
            tc.tile_pool(name="stage", bufs=2) as spool,
            tc.tile_pool(name="q16", bufs=2) as qpool,
            tc.tile_pool(name="xT", bufs=3) as xtpool,
            tc.tile_pool(name="tps", bufs=3, space="PSUM") as tppool,
            tc.tile_pool(name="mmps", bufs=4, space="PSUM") as mmpool,
            tc.tile_pool(name="osb", bufs=2) as opool,
        ):
            ident = cpool.tile([P, P], f16)
            make_identity(nc, ident)
            bias_bc = cpool.tile([P, OPER], f32)
            nc.sync.dma_start(bias_bc, b_d[None, :].to_broadcast((P, OPER)))

            # Quantized transposed W slices, SBUF resident.
            wT = [wpool.tile([P, KT, NOC], f16, name=f"wT{oc}")
                  for oc in range(OC)]
            wstage = wstgpool.tile([P, KT, NOC], f16)

            # ---- W preamble: ACT-queue loads, DVE quantize, PE transpose.
            for oc in range(OC):
                for oi in range(OT // OC):
                    ot = oc * (OT // OC) + oi
                    wq = wqpool.tile([P, D], f16, tag="wq16")
                    for h in range(2):
                        wl = wlpool.tile([P, HW_], f32, tag="wload")
                        nc.scalar.dma_start(
                            wl, w_d[ot * P:(ot + 1) * P,
                                    h * HW_:(h + 1) * HW_])
                        nc.vector.tensor_scalar(wl, wl, 100.0, _MAGIC,
                                                mybir.AluOpType.mult,
                                                mybir.AluOpType.add)
                        nc.vector.tensor_scalar(
                            wq[:, h * HW_:(h + 1) * HW_], wl,
                            -_MAGIC, 0.0,
                            mybir.AluOpType.add, mybir.AluOpType.add)
                    for g in range(KT // 8):
                        tp = tppool.tile([P, 8, P], f16, tag="tps")
                        for j in range(8):
                            kt = g * 8 + j
                            nc.tensor.transpose(tp[:, j, :],
                                                wq[:, kt * P:(kt + 1) * P],
                                                ident)
                        nc.vector.tensor_copy(
                            wstage[:, g * 8:(g + 1) * 8, oi * P:(oi + 1) * P],
                            tp)
                nc.vector.tensor_copy(wT[oc], wstage)

            # ---- Main loop over x stripes: PE-transposed x.
            for bt in range(BT):
                st = spool.tile([P, D], f32, tag="stage")
                nc.sync.dma_start(st, x_d[bt * P:(bt + 1) * P, :])
                nc.vector.tensor_scalar(st, st, 100.0, _MAGIC,
                                        mybir.AluOpType.mult,
                                        mybir.AluOpType.add)
                q = qpool.tile([P, D], f16, tag="q16")
                nc.scalar.activation(q, st, Copy, bias=-_MAGIC, scale=1.0)
                xT = xtpool.tile([P, KT, P], f16, tag="xT")
                for g in range(KT // 8):
                    tp = tppool.tile([P, 8, P], f16, tag="tps")
                    for j in range(8):
                        kt = g * 8 + j
                        nc.tensor.transpose(tp[:, j, :],
                                            q[:, kt * P:(kt + 1) * P], ident)
                    nc.vector.tensor_copy(xT[:, g * 8:(g + 1) * 8, :], tp)

                ob = opool.tile([P, OPER], f32, tag="osb")
                for oc in range(OC):
                    ps = mmpool.tile([P, NOC], f32, name="ps", tag="mmps")
                    for kt in range(KT):
                        nc.tensor.matmul(ps, xT[:, kt, :], wT[oc][:, kt, :],
                                         start=(kt == 0), stop=(kt == KT - 1))
                    nc.scalar.activation(ob[:, oc * NOC:(oc + 1) * NOC],
                                         ps, Copy, bias=0.0, scale=1e-4)
                nc.vector.tensor_add(ob, ob, bias_bc)
                nc.gpsimd.dma_start(o_d[bt * P:(bt + 1) * P, :], ob)

    nc.compile()
    return nc


def _get_nc(BPER=_BPER, D=_D, OPER=_OPER):
    key = (BPER, D, OPER)
    if key not in _nc_cache:
        _nc_cache[key] = _build(BPER, D, OPER)
    return _nc_cache[key]


def _make_in_maps(x, W, b):
    BPER = x.shape[0] // _R
    OPER = W.shape[0] // _C
    in_maps = []
    for c in range(_NCORES):
        r, q = divmod(c, _C)
        in_maps.append({
            "x": np.ascontiguousarray(x[r * BPER:(r + 1) * BPER]),
            "w": np.ascontiguousarray(W[q * OPER:(q + 1) * OPER]),
            "b": np.ascontiguousarray(b[q * OPER:(q + 1) * OPER]),
        })
    return in_maps


def _assemble(outs):
    rows = []
    for r in range(_R):
        rows.append(np.concatenate(outs[r * _C:(r + 1) * _C], axis=1))
    return np.concatenate(rows, axis=0)


def _run(x, W, b, trace=False):
    from concourse.bass_utils import run_bass_kernel_spmd

    BPER = x.shape[0] // _R
    D = x.shape[1]
    OPER = W.shape[0] // _C
    nc = _get_nc(BPER, D, OPER)
    in_maps = _make_in_maps(x, W, b)
    res = run_bass_kernel_spmd(nc, in_maps, core_ids=list(range(_NCORES)),
                               trace=trace)
    out = _assemble([res.results[c]["out"] for c in range(_NCORES)])
    return out, res


def kernel(x=None, W=None, b=None):
    x = np.ascontiguousarray(np.asarray(x, dtype=np.float32))
    W = np.ascontiguousarray(np.asarray(W, dtype=np.float32))
    b = np.ascontiguousarray(np.asarray(b, dtype=np.float32))
    out, _ = _run(x, W, b, trace=False)
    return out
